# revision 64
# baseline (speedup 1.0000x reference)
"""Trainium2 Bass kernel for nn_LocallyDense: gather -> 16 group-GEMMs -> Conv1D(k=32) -> LeakyReLU.

Strategy: data-parallel over batch (32 -> 4 per core on 8 cores).
Host staging applies the idx permutation + transpose so the device sees dense
GEMMs only.

  stage 1 (bf16): h[d, (n,b)] = sum_kt W[g]^T x_perm, per group g; the fp32
    PSUM result is stored as bf16 h.
  stage 2: depth-2 Winograd/Karatsuba F(2,2) over the 32 conv taps.
    Tap pairs are recursively split (3 products per pair instead of 4), giving
    9 product streams (AA..CC) of 8 taps each over j2 = t//4 columns. Products
    run as fp8 DoubleRow matmuls (e4m3 hi + e5m2 lo corrections); conv weights
    pre-scaled by 64 (LeakyReLU is positively homogeneous, 1/64 folded out on
    the host). Leaf sequences are computed on DVE/Act from bf16 h:
      D[t] = h[t]-h[t+1]
      AA=h[4m+3]        AB=h[4m+1]-h[4m+3]  AC=h[4m+3]-h[4m+5]
      BA=h[4m+2]-h[4m+3] BB=D[4m]-D[4m+2]   BC=D[4m+2]-D[4m+4]
      CA=h[4m+3]-h[4m+4] CB=D[4m+1]-D[4m+3] CC=D[4m+3]-D[4m+5]
    Reconstruction (per j2): y[4j2+0] = AA+AB+BA+BB
                             y[4j2+1] = AA+AB-CA-CB
                             y[4j2+2] = AA-AC+BA-BC
                             y[4j2+3] = AA-AC-CA+CC
    The 9 streams accumulate in shared PSUM banks (one start=True zeroes the
    whole 2KB bank; verified on hw). Dropped lo-corrections (h-side of CC,
    4 w-side taps of CC, 2 of CB) trade 1.83e-2 rel err for time (budget
    2e-2; errsim.py predicts device error to 3 digits).
  epilogue: adds on DVE/Pool, LeakyReLU on Act, bf16 out, host transpose.

has_bias (not hit by the harness inputs: biases are zero) falls back to the
direct 3-term scheme with a host-precomputed bias table.
"""
import numpy as np
import ml_dtypes

import concourse.bass as bass
import concourse.mybir as mybir
import concourse.tile as tile
from concourse.alu_op_type import AluOpType
from concourse import bacc
from concourse.bass_utils import run_bass_kernel_spmd

B, N, F, G, S, D = 32, 1024, 512, 16, 64, 256
KC, O = 32, 512            # conv taps, conv out channels
T = N - KC + 1             # 993 valid conv outputs
NCORES = 8
BPC = B // NCORES          # batches per core
FKT = F // 128             # k-tiles over F
NEG_SLOPE = 0.2
WSCALE = 64.0              # conv-weight pre-scale into e4m3 range (power of 2)

NS = 9                     # wino streams AA AB AC BA BB BC CA CB CC
PT = 8                     # taps per stream
NL = 256                   # leaf length allocated
J2 = 249                   # j2 columns (y phase 0)
NB = N * BPC

STREAMS = ["AA", "AB", "AC", "BA", "BB", "BC", "CA", "CB", "CC"]
# per-stream max valid leaf index + 1
LEAF_LEN = {"AA": 256, "AB": 256, "AC": 255, "BA": 256, "BB": 256,
            "BC": 255, "CA": 255, "CB": 255, "CC": 255}
# drop plan (errsim wino_final: rel 1.71e-2): h-side lo of CC dropped fully,
# w-side lo of CC taps 0-3 dropped
H_KEEP = {s: list(range(PT)) for s in STREAMS}
H_KEEP["CC"] = []
W_KEEP = {s: list(range(PT)) for s in STREAMS}
W_KEEP["CC"] = [4, 5, 6, 7]
W_KEEP["CB"] = [2, 3, 4, 5, 6, 7]

F32 = mybir.dt.float32
BF16 = mybir.dt.bfloat16
E4 = mybir.dt.float8e4
E5 = mybir.dt.float8e5
E4np = ml_dtypes.float8_e4m3
E5np = ml_dtypes.float8_e5m2
BFnp = ml_dtypes.bfloat16

TRACE = False              # test.py flips this to get a profile
_cache = {}

DR = mybir.MatmulPerfMode.DoubleRow
LRELU = mybir.ActivationFunctionType.Prelu


def _build_wino():
    nc = bacc.Bacc("TRN2", target_bir_lowering=False, debug=False,
                   num_devices=NCORES)
    # x layout: col = (n*BPC + b)*FKT + kt
    xp_d = nc.dram_tensor("xp", [128, N * BPC * FKT], BF16,
                          kind="ExternalInput").ap()
    w_d = nc.dram_tensor("w", [4, 128, 4 * FKT * D], BF16,
                         kind="ExternalInput").ap()
    cwh_d = nc.dram_tensor("cwh", [4, 128, NS * PT * 2 * 128], E4,
                           kind="ExternalInput").ap()
    cwl_d = nc.dram_tensor("cwl", [4, 128, NS * PT * 2 * 128], E5,
                           kind="ExternalInput").ap()
    y_d = nc.dram_tensor("y", [BPC, O, T], BF16, kind="ExternalOutput").ap()

    with tile.TileContext(nc) as tc:
        with tc.tile_pool(name="x", bufs=4) as p_x, \
             tc.tile_pool(name="wg", bufs=4) as p_w, \
             tc.tile_pool(name="hd", bufs=1) as p_hd, \
             tc.tile_pool(name="leaf", bufs=1) as p_leaf, \
             tc.tile_pool(name="tmp", bufs=2) as p_tmp, \
             tc.tile_pool(name="cw", bufs=2) as p_cw, \
             tc.tile_pool(name="ep", bufs=2) as p_ep, \
             tc.tile_pool(name="yout", bufs=4) as p_out, \
             tc.tile_pool(name="ps1", bufs=2, space="PSUM") as p_ps1, \
             tc.tile_pool(name="ps2", bufs=6, space="PSUM") as p_ps2:

            # ---- tiles ----
            # x streamed through quarter-sized buffers (4 groups each)
            XQ = FKT * N * BPC // 4
            xq_sb = [p_x.tile([128, XQ], BF16, tag="x", name=f"xq{q}",
                              bufs=2) for q in range(4)]
            x_v = [t[:].rearrange("p (c kt) -> p c kt", kt=FKT)
                   for t in xq_sb]
            w_sb = [p_w.tile([128, 4 * FKT * D], BF16, tag="w", name=f"w{wt}")
                    for wt in range(4)]
            # h, D: [p, dh, n, b]
            h_sb = p_hd.tile([128, 2 * NB], BF16, tag="h", name="h_sb")
            d_sb = p_hd.tile([128, 2 * NB], BF16, tag="d", name="d_sb")
            hh = h_sb[:].rearrange("p (dh n b) -> p dh n b", dh=2, b=BPC)
            dd = d_sb[:].rearrange("p (dh n b) -> p dh n b", dh=2, b=BPC)
            # stride-4 views: n = 4*m + r
            hh4 = h_sb[:].rearrange("p (dh m r b) -> p dh m r b", dh=2, r=4,
                                    b=BPC)
            dd4 = d_sb[:].rearrange("p (dh m r b) -> p dh m r b", dh=2, r=4,
                                    b=BPC)
            # leaf streams: [p, s, dh, m, b]
            lf_hi = p_leaf.tile([128, NS * 2 * NL * BPC], E4, tag="lh",
                                name="lf_hi")
            # lo leaf: 8 streams only (CC's h-correction is dropped)
            lf_lo = p_leaf.tile([128, (NS - 1) * 2 * NL * BPC], E5, tag="ll",
                                name="lf_lo")
            lh = lf_hi[:].rearrange("p (s dh m b) -> p s dh m b", s=NS, dh=2,
                                    b=BPC)
            ll = lf_lo[:].rearrange("p (s dh m b) -> p s dh m b", s=NS - 1,
                                    dh=2, b=BPC)
            # conv weights, double-buffered by mo
            cwh_sb = [p_cw.tile([128, NS * PT * 2 * 128], E4, tag="cwh",
                                name=f"cwh{m}") for m in range(4)]
            cwl_sb = [p_cw.tile([128, NS * PT * 2 * 128], E5, tag="cwl",
                                name=f"cwl{m}") for m in range(4)]
            cwh_v = [t[:].rearrange("p (s tap dh o) -> p s tap dh o", s=NS,
                                    tap=PT, dh=2) for t in cwh_sb]
            cwl_v = [t[:].rearrange("p (s tap dh o) -> p s tap dh o", s=NS,
                                    tap=PT, dh=2) for t in cwl_sb]

            # ---- DMA emission (order == priority) ----
            def xchunk(q):
                nc.sync.dma_start(xq_sb[q][:], xp_d[:, q * XQ:(q + 1) * XQ])

            WH = 2 * FKT * D

            def wchunk(g2):
                wt, half = g2 // 2, g2 % 2
                nc.sync.dma_start(w_sb[wt][:, half * WH:(half + 1) * WH],
                                  w_d[wt, :, half * WH:(half + 1) * WH])

            # cw tiles are DMAed in 3-stream chunks so the conv can chase
            # the transfers (the DMA engine path is serial in practice).
            CWC = 3 * PT * 2 * 128                     # 3-stream chunk cols

            def cwchunk(kind, mo, c):
                src = cwh_d if kind == "h" else cwl_d
                dst = (cwh_sb if kind == "h" else cwl_sb)[mo]
                nc.sync.dma_start(dst[:, c * CWC:(c + 1) * CWC],
                                  src[mo, :, c * CWC:(c + 1) * CWC])

            # need-ordered: stage-1 feed (x, w) interleaved with mo0 conv
            # weights; later mo weights are emitted inside the conv loop so
            # y-out DMAs aren't stuck behind them.
            # cwh/cwl chunks interleaved per 3-stream group so mo0's T1
            # banks (stream-aligned) can close as soon as their chunk lands
            # first x quarter split in halves: group 0 is fed sooner
            nc.sync.dma_start(xq_sb[0][:, :XQ // 2], xp_d[:, :XQ // 2])
            wchunk(0)
            nc.sync.dma_start(xq_sb[0][:, XQ // 2:], xp_d[:, XQ // 2:XQ])
            wchunk(1)
            xchunk(1); wchunk(2); wchunk(3)           # groups 4-7
            cwchunk("h", 0, 0); cwchunk("l", 0, 0)
            xchunk(2); wchunk(4); wchunk(5)           # groups 8-11
            cwchunk("h", 0, 1); cwchunk("l", 0, 1)
            cwchunk("h", 0, 2); cwchunk("l", 0, 2)
            xchunk(3); wchunk(6); wchunk(7)           # groups 12-15
            for c in range(3):
                cwchunk("h", 1, c)

            # ---- PE warm-up ----
            warm_sb = p_hd.tile([128, 64], BF16, tag="warm", name="warm_sb",
                                bufs=1)
            nc.vector.memset(warm_sb[:], 0)

            def pewarm(n, label):
                ps = p_ps1.tile([128, BPC * S], F32, tag="ps1",
                                name=f"warm_{label}")
                for i in range(n):
                    nc.tensor.matmul(ps[:64, :64], warm_sb[:, :64],
                                     warm_sb[:, :64], start=True, stop=True)

            # ---- stage 1 ----
            def stage1(g):
                for m in range(2):
                    ps = p_ps1.tile([128, BPC * S], F32, tag="ps1",
                                    name=f"ps1_{g}_{m}")
                    for kt in range(FKT):
                        base = (g % 4) * FKT * D + kt * D + m * 128
                        lhsT = w_sb[g // 4][:, base: base + 128]
                        gq = g % 4
                        rhs = x_v[g // 4][:, gq * S * BPC:(gq + 1) * S * BPC,
                                          kt]
                        nc.tensor.matmul(ps[:], lhsT, rhs,
                                         start=(kt == 0), stop=(kt == FKT - 1))
                    psv = ps[:].rearrange("p (n b) -> p n b", b=BPC)
                    nc.scalar.copy(hh[:, m, g * S:(g + 1) * S, :], psv)

            # ---- transforms: D + 9 leaf streams, chunked over m ----
            # chunk c: m in [m0, m1); D over t in [4*m0, min(4*m1+6, 1023))
            def transforms(m0, m1):
                t0, t1 = 4 * m0, min(4 * m1 + 6, N - 1)
                nc.vector.tensor_tensor(dd[:, :, t0:t1, :],
                                        hh[:, :, t0:t1, :],
                                        hh[:, :, t0 + 1:t1 + 1, :],
                                        AluOpType.subtract)
                mw = m1 - m0

                def mview(v4, r, off=0):
                    # v4[:, :, m0+off : m1+off, r, :] clipped to valid range
                    return v4[:, :, m0 + off:m1 + off, r, :]

                # leaf sub specs: stream -> (src4, r_a, off_a, r_b, off_b)
                subs = {
                    "AB": (hh4, 1, 0, 3, 0), "AC": (hh4, 3, 0, 1, 1),
                    "BA": (hh4, 2, 0, 3, 0), "CA": (hh4, 3, 0, 0, 1),
                    "BB": (dd4, 0, 0, 2, 0), "BC": (dd4, 2, 0, 0, 1),
                    "CB": (dd4, 1, 0, 3, 0), "CC": (dd4, 3, 0, 1, 1),
                }
                # walrus vector/stt ops allow at most 2 free dims; the
                # stride-4 h/D views (dh, m, b) don't collapse, so ops that
                # read them are emitted once per dh half.
                # Engine split: subs on DVE; lo-corrections on Pool (SBUF
                # only, Pool is otherwise idle); hi casts on Act except the
                # first chunk (keeps the Act queue clear for stage-1 psum
                # copies, whose pool has only 2 buffers).
                def hi_eng_copy(out, in_):
                    nc.scalar.copy(out, in_)

                for s in STREAMS:
                    si = STREAMS.index(s)
                    me = min(m1, LEAF_LEN[s])
                    if me <= m0:
                        continue
                    w = me - m0
                    hi = lh[:, si, :, m0:me, :]
                    lo = (ll[:, si, :, m0:me, :] if si < NS - 1 else None)
                    if s == "AA":
                        for dh in range(2):
                            src = hh4[:, dh, m0:me, 3, :]
                            hi_eng_copy(hi[:, dh], src)
                            # split the lo halves across DVE and Pool
                            if dh == 0:
                                nc.vector.scalar_tensor_tensor(
                                    lo[:, dh], src, 1.0, hi[:, dh],
                                    AluOpType.mult, AluOpType.subtract)
                            else:
                                nc.gpsimd.tensor_tensor(
                                    lo[:, dh], src, hi[:, dh],
                                    AluOpType.subtract)
                        continue
                    v4, ra, oa, rb, ob = subs[s]
                    if s == "CC":
                        # h-corr dropped: quantize straight to e4m3
                        for dh in range(2):
                            nc.vector.tensor_tensor(
                                hi[:, dh], v4[:, dh, m0 + oa:me + oa, ra, :],
                                v4[:, dh, m0 + ob:me + ob, rb, :],
                                AluOpType.subtract)
                        continue
                    tmp = p_tmp.tile([128, 2 * 120 * BPC], BF16, tag="tmp",
                                     name=f"tmp_{s}_{m0}")
                    tv = tmp[:].rearrange("p (dh m b) -> p dh m b", dh=2,
                                          b=BPC)[:, :, :w, :]
                    for dh in range(2):
                        nc.vector.tensor_tensor(
                            tv[:, dh], v4[:, dh, m0 + oa:me + oa, ra, :],
                            v4[:, dh, m0 + ob:me + ob, rb, :],
                            AluOpType.subtract)
                    hi_eng_copy(hi, tv)
                    nc.vector.scalar_tensor_tensor(
                        lo[:, 0], tv[:, 0], 1.0, hi[:, 0],
                        AluOpType.mult, AluOpType.subtract)
                    nc.gpsimd.tensor_tensor(lo[:, 1], tv[:, 1], hi[:, 1],
                                            AluOpType.subtract)

            # ---- conv tiles ----
            # tile = (q0, w): j2 range [q0, q0+w). Stream s covers width
            # w if q0+w <= 248 else (w if y0-only stream else w-1)
            WIDE = {"AA", "AB", "BA", "BB"}     # used at j2 = 248 (phase 0)

            def swidth(s, q0, w):
                if q0 + w >= J2 and s not in WIDE:
                    return w - 1
                return w

            def conv_tile(mo, bb, q0, w, banks):
                """banks: list of lists of stream indices sharing one psum
                bank; stream k at col offset slot*(idx in bank)."""
                slot = 512 // max(len(bk) for bk in banks)
                pss = []
                for bi, bk in enumerate(banks):
                    ps = p_ps2.tile([128, 512], F32, tag="ps2",
                                    name=f"ps2_{mo}_{bb}_{q0}_{bi}")
                    ops = []
                    for k, si in enumerate(bk):
                        s = STREAMS[si]
                        ws_ = swidth(s, q0, w)
                        off = k * slot
                        for tap in range(PT):
                            ops.append((si, tap, "m", off, ws_))
                        for tap in H_KEEP[s]:
                            ops.append((si, tap, "h", off, ws_))
                        for tap in W_KEEP[s]:
                            ops.append((si, tap, "w", off, ws_))
                    for i, (si, tap, kind, off, ws_) in enumerate(ops):
                        if kind == "m":
                            l = cwh_v[mo][:, si, tap]
                            r = lh[:, si, :, q0 + tap:q0 + tap + ws_, bb]
                        elif kind == "h":
                            l = cwh_v[mo][:, si, tap]
                            r = ll[:, si, :, q0 + tap:q0 + tap + ws_, bb]
                        else:
                            l = cwl_v[mo][:, si, tap]
                            r = lh[:, si, :, q0 + tap:q0 + tap + ws_, bb]
                        nc.tensor.matmul(ps[:, off:off + ws_], l, r,
                                         start=(i == 0),
                                         stop=(i == len(ops) - 1),
                                         perf_mode=DR)
                    pss.append(ps)
                return pss, slot

            def epilogue_A(mo, bb, q0, w, banks, pss, slot, fast_tail=False):
                """PSUM-reading phase: copies of shared streams + partial
                adds. Running this promptly releases the unit's psum banks;
                the finals/activation (phase B) can lag behind."""
                def pr(si):
                    for bi, bk in enumerate(banks):
                        if si in bk:
                            off = bk.index(si) * slot
                            return pss[bi][:, off:off + swidth(
                                STREAMS[si], q0, w)]
                    raise KeyError(si)

                wn = w - 1 if q0 + w >= J2 else w   # narrow width (phases 1-3)
                AA, AB, AC = pr(0), pr(1), pr(2)
                BA, BB, BC = pr(3), pr(4), pr(5)
                CA, CB, CC = pr(6), pr(7), pr(8)
                # vector ops may read only ONE psum operand: copy the
                # shared streams (AA, BA, CA) to SBUF first, then each
                # partial add reads one psum + one sbuf input. Slot reuse:
                # u->t1, v->t3, w2->t5, then y0->t3, y1->t5, y2->t4,
                # y3->t6 (6 slots total).
                tp = p_ep.tile([128, 6 * 128], F32, tag="tp",
                               name=f"tp_{mo}_{bb}_{q0}")
                sl = [tp[:, k * 128:k * 128 + w] for k in range(6)]
                u, v, w2 = sl[0], sl[2], sl[4][:, :wn]
                nc.scalar.copy(u, AA)
                nc.scalar.copy(v, BA)
                nc.scalar.copy(w2, CA)
                t2 = sl[1][:, :wn]; t4 = sl[3][:, :wn]; t6 = sl[5][:, :wn]
                nc.vector.tensor_tensor(t2, u[:, :wn], AC, AluOpType.subtract)
                t1 = u
                nc.vector.tensor_tensor(t1, u, AB, AluOpType.add)
                nc.vector.tensor_tensor(t4, v[:, :wn], BC, AluOpType.subtract)
                t3 = v
                nc.vector.tensor_tensor(t3, v, BB, AluOpType.add)
                nc.vector.tensor_tensor(t6, w2, CC, AluOpType.subtract)
                t5 = w2
                nc.vector.tensor_tensor(t5, w2, CB, AluOpType.add)
                return (q0, w, wn, t1, t2, t3, t4, t5, t6, fast_tail)

            def epilogue_B(ctx, y_sb):
                q0, w, wn, t1, t2, t3, t4, t5, t6, fast_tail = ctx
                y0, y1, y2, y3 = t3, t5, t4, t6
                feng = nc.vector if fast_tail else nc.gpsimd
                feng.tensor_tensor(y0, t1, t3, AluOpType.add)
                feng.tensor_tensor(y1, t1[:, :wn], t5, AluOpType.subtract)
                nc.vector.tensor_tensor(y2, t2, t4, AluOpType.add)
                nc.vector.tensor_tensor(y3, t2, t6, AluOpType.subtract)
                yv = y_sb[:].rearrange("p (j2 r) -> p j2 r", r=4)
                nc.scalar.activation(yv[:, q0:q0 + w, 0], y0, LRELU,
                                     alpha=NEG_SLOPE)
                nc.scalar.activation(yv[:, q0:q0 + wn, 1], y1, LRELU,
                                     alpha=NEG_SLOPE)
                nc.scalar.activation(yv[:, q0:q0 + wn, 2], y2, LRELU,
                                     alpha=NEG_SLOPE)
                nc.scalar.activation(yv[:, q0:q0 + wn, 3], y3, LRELU,
                                     alpha=NEG_SLOPE)

            BANKS2 = [[0, 1, 2, 3, 4, 5, 6, 7], [8]]          # w <= 64
            BANKS3 = [[0, 1, 2], [3, 4, 5], [6, 7, 8]]        # chunk-aligned
            BANKS4 = [[0, 1, 2, 3], [4, 5, 6, 7], [8]]        # w <= 128
            BANKS5 = [[0, 1, 2, 3, 4], [5, 6, 7, 8]]          # w <= 96

            # phase-B (finals + activation) lags one sub-unit behind so the
            # next unit's psum-releasing phase-A ops aren't queued behind
            # cross-engine finals on the in-order engines.
            pendB = []

            def flushB(keep):
                while len(pendB) > keep:
                    pendB.pop(0)()

            def conv_unit(mo, bb, q0, w, banks, y_sb, fast_tail=False,
                          post=None):
                pss, slot = conv_tile(mo, bb, q0, w, banks)
                ctx = epilogue_A(mo, bb, q0, w, banks, pss, slot, fast_tail)

                def B():
                    epilogue_B(ctx, y_sb)
                    if post is not None:
                        post()
                pendB.append(B)
                flushB(1)

            # ---- schedule ----
            # transforms(c) is emitted after the last stage-1 group whose h it
            # needs, so the Act queue isn't blocked ahead of stage-1 copies.
            # pewarm bursts pad the DMA-starved windows (keeps the PE p-state
            # ramped so post-stall matmuls run at full clock).
            pewarm(70, "boot")
            for g in range(3):
                stage1(g)
            transforms(0, 40)          # needs h < 166  (g0-2)
            for g in range(3, 5):
                stage1(g)
            transforms(40, 72)         # needs h < 294  (g0-4)
            pewarm(20, "w0")
            for g in range(5, 7):
                stage1(g)
            transforms(72, 104)        # needs h < 422  (g0-6)
            for g in range(7, 9):
                stage1(g)
            transforms(104, 136)       # needs h < 550  (g0-8)
            pewarm(25, "w1")
            for g in range(9, 13):
                stage1(g)
            pewarm(25, "w2")
            for g in range(13, G):
                stage1(g)

            pewarm(40, "bridge")

            def ydma(mo, bb, y_sb):
                nc.sync.dma_start(y_d[bb, mo * 128:(mo + 1) * 128, :],
                                  y_sb[:, :T])

            # Unit schedule: T1 covers j2 [0,128) (two 64-wide units for
            # mo0, gated on transform chunks 0/1), T2 covers [128,249).
            # T2 units lag two T1 units behind so each unit's epilogue
            # drain (and psum-bank release) overlaps the next units'
            # matmuls, including across mo boundaries. mo0's last
            # transform chunk is emitted in 3 pieces between units so the
            # in-order DVE/Act queues interleave transform and epilogue
            # work; later cw DMAs are fed after the first unit of the
            # preceding mo.
            y_tiles = {}

            def emit_T1(mo, bb):
                y_sb = p_out.tile([128, 996], BF16, tag="y",
                                  name=f"y_{mo}_{bb}")
                y_tiles[(mo, bb)] = y_sb
                if mo == 0:
                    conv_unit(mo, bb, 0, 64, BANKS3, y_sb)
                    conv_unit(mo, bb, 64, 64, BANKS3, y_sb)
                    if bb < 3:
                        transforms(136 + 40 * bb, 176 + 40 * bb)
                    if bb == 0:
                        for c in range(3):
                            cwchunk("l", 1, c)
                else:
                    conv_unit(mo, bb, 0, 128, BANKS4, y_sb)
                    if bb == 0 and mo < 3:
                        for c in range(3):
                            cwchunk("h", mo + 1, c)
                            cwchunk("l", mo + 1, c)

            def emit_T2(mo, bb):
                y_sb = y_tiles[(mo, bb)]
                if mo == 3 and bb == BPC - 1:
                    # short tail: a small fast final tile + split y DMA
                    conv_unit(mo, bb, 128, 96, BANKS5, y_sb,
                              post=lambda: nc.sync.dma_start(
                                  y_d[bb, mo * 128:(mo + 1) * 128, :896],
                                  y_sb[:, :896]))
                    conv_unit(mo, bb, 224, 25, BANKS2, y_sb, fast_tail=True,
                              post=lambda: nc.sync.dma_start(
                                  y_d[bb, mo * 128:(mo + 1) * 128, 896:],
                                  y_sb[:, 896:T]))
                else:
                    conv_unit(mo, bb, 128, 121, BANKS4, y_sb,
                              post=lambda m=mo, b=bb, t=y_sb: ydma(m, b, t))

            # mo0's T1 units all run first (T2 needs the late transform
            # chunk); from mo1 on, T2 units lag two units behind.
            t2pend = []
            for mo in range(4):
                for bb in range(BPC):
                    emit_T1(mo, bb)
                    t2pend.append((mo, bb))
                    if mo == 3 and bb > 0:
                        # drain faster in the last mo so the final
                        # epilogue chains don't bunch at the end
                        emit_T2(*t2pend.pop(0))
                        if len(t2pend) > 1:
                            emit_T2(*t2pend.pop(0))
                    elif mo > 0 and len(t2pend) > 2:
                        emit_T2(*t2pend.pop(0))
                if mo == 0:
                    emit_T2(*t2pend.pop(0))
                    emit_T2(*t2pend.pop(0))
            for p in t2pend:
                emit_T2(*p)
            flushB(0)

    nc.compile()
    return nc


def _host_stage_wino(x, idx, W, conv_w):
    idx_flat = idx.reshape(-1).astype(np.int64)
    xg = x[:, idx_flat, :].astype(BFnp)                       # [B, N, F]
    xp = np.ascontiguousarray(
        xg.transpose(2, 1, 0).reshape(FKT, 128, N, NCORES, BPC)
        .transpose(3, 1, 2, 4, 0)).reshape(NCORES, 128, N * BPC * FKT)

    wq = np.ascontiguousarray(
        W.astype(BFnp).reshape(4, 4, FKT, 128, D).transpose(0, 3, 1, 2, 4)
    ).reshape(4, 128, 4 * FKT * D)

    # wino leaf weights: streams AA AB AC BA BB BC CA CB CC
    ws = (conv_w * np.float32(WSCALE)).astype(np.float32)     # [32, D, O]
    lvl1 = {"A": ws[0::2] + ws[1::2], "B": ws[0::2], "C": ws[1::2]}
    streams = []
    for s1 in "ABC":
        u = lvl1[s1]
        streams += [u[0::2] + u[1::2], u[0::2], u[1::2]]
    cws = np.stack(streams)                                   # [9, 8, D, O]
    # [s, tap, dh, p, mo, o] -> [mo, p, s, tap, dh, o]
    cwt = cws.reshape(NS, PT, 2, 128, 4, 128).transpose(4, 3, 0, 1, 2, 5)
    cwh = cwt.astype(E4np)
    cwl = (cwt - cwh.astype(np.float32)).astype(E5np)
    cwh = np.ascontiguousarray(cwh).reshape(4, 128, NS * PT * 2 * 128)
    cwl = np.ascontiguousarray(cwl).reshape(4, 128, NS * PT * 2 * 128)
    return xp, wq, cwh, cwl


def kernel(x, idx, W, b, conv_w, conv_b):
    x = np.asarray(x); idx = np.asarray(idx); W = np.asarray(W)
    b = np.asarray(b); conv_w = np.asarray(conv_w); conv_b = np.asarray(conv_b)
    has_bias = bool(np.any(b) or np.any(conv_b))
    if has_bias:
        # the harness always passes zero biases (spec fill=zeros); keep a
        # self-contained exact fallback for the general case
        xg = x[:, idx.reshape(-1).astype(np.int64), :]
        h = np.einsum('bgsf,gfd->bgsd',
                      xg.reshape(B, G, S, F).astype(np.float64),
                      W.astype(np.float64), optimize=True) + b[None, :, None]
        h = h.reshape(B, N, D)
        y = np.zeros((B, T, O), np.float64)
        for k in range(KC):
            y += np.einsum('btd,do->bto', h[:, k:k + T],
                           conv_w[k].astype(np.float64), optimize=True)
        y += conv_b[None, None, :]
        return np.where(y >= 0, y, NEG_SLOPE * y).astype(np.float32)

    if "wino" not in _cache:
        _cache["wino"] = _build_wino()
        _cache["nc"] = _cache["wino"]   # for test.py's TimelineSim hook
    nc = _cache["wino"]

    xp, wq, cwh, cwl = _host_stage_wino(x, idx, W, conv_w)
    in_maps = [{"xp": xp[c], "w": wq, "cwh": cwh, "cwl": cwl}
               for c in range(NCORES)]

    res = run_bass_kernel_spmd(nc, in_maps, core_ids=list(range(NCORES)),
                               trace=TRACE)
    if TRACE and res.exec_time_ns is not None:
        print(f"HW exec time: {res.exec_time_ns} ns")
    y = np.stack([r["y"] for r in res.results])       # [NC, BPC, O, T] bf16
    y = y.reshape(B, O, T).transpose(0, 2, 1).astype(np.float32)
    return np.ascontiguousarray(y * np.float32(1.0 / WSCALE))


# revision 67
# speedup vs baseline: 1.0022x; 1.0022x over previous
"""Trainium2 Bass kernel for nn_LocallyDense: gather -> 16 group-GEMMs -> Conv1D(k=32) -> LeakyReLU.

Strategy: data-parallel over batch (32 -> 4 per core on 8 cores).
Host staging applies the idx permutation + transpose so the device sees dense
GEMMs only.

  stage 1 (bf16): h[d, (n,b)] = sum_kt W[g]^T x_perm, per group g; the fp32
    PSUM result is stored as bf16 h.
  stage 2: depth-2 Winograd/Karatsuba F(2,2) over the 32 conv taps.
    Tap pairs are recursively split (3 products per pair instead of 4), giving
    9 product streams (AA..CC) of 8 taps each over j2 = t//4 columns. Products
    run as fp8 DoubleRow matmuls (e4m3 hi + e5m2 lo corrections); conv weights
    pre-scaled by 64 (LeakyReLU is positively homogeneous, 1/64 folded out on
    the host). Leaf sequences are computed on DVE/Act from bf16 h:
      D[t] = h[t]-h[t+1]
      AA=h[4m+3]        AB=h[4m+1]-h[4m+3]  AC=h[4m+3]-h[4m+5]
      BA=h[4m+2]-h[4m+3] BB=D[4m]-D[4m+2]   BC=D[4m+2]-D[4m+4]
      CA=h[4m+3]-h[4m+4] CB=D[4m+1]-D[4m+3] CC=D[4m+3]-D[4m+5]
    Reconstruction (per j2): y[4j2+0] = AA+AB+BA+BB
                             y[4j2+1] = AA+AB-CA-CB
                             y[4j2+2] = AA-AC+BA-BC
                             y[4j2+3] = AA-AC-CA+CC
    The 9 streams accumulate in shared PSUM banks (one start=True zeroes the
    whole 2KB bank; verified on hw). Dropped lo-corrections (h-side of CC,
    4 w-side taps of CC, 2 of CB) trade 1.83e-2 rel err for time (budget
    2e-2; errsim.py predicts device error to 3 digits).
  epilogue: adds on DVE/Pool, LeakyReLU on Act, bf16 out, host transpose.

has_bias (not hit by the harness inputs: biases are zero) falls back to the
direct 3-term scheme with a host-precomputed bias table.
"""
import numpy as np
import ml_dtypes

import concourse.bass as bass
import concourse.mybir as mybir
import concourse.tile as tile
from concourse.alu_op_type import AluOpType
from concourse import bacc
from concourse.bass_utils import run_bass_kernel_spmd

B, N, F, G, S, D = 32, 1024, 512, 16, 64, 256
KC, O = 32, 512            # conv taps, conv out channels
T = N - KC + 1             # 993 valid conv outputs
NCORES = 8
BPC = B // NCORES          # batches per core
FKT = F // 128             # k-tiles over F
NEG_SLOPE = 0.2
WSCALE = 64.0              # conv-weight pre-scale into e4m3 range (power of 2)

NS = 9                     # wino streams AA AB AC BA BB BC CA CB CC
PT = 8                     # taps per stream
NL = 256                   # leaf length allocated
J2 = 249                   # j2 columns (y phase 0)
NB = N * BPC

STREAMS = ["AA", "AB", "AC", "BA", "BB", "BC", "CA", "CB", "CC"]
# per-stream max valid leaf index + 1
LEAF_LEN = {"AA": 256, "AB": 256, "AC": 255, "BA": 256, "BB": 256,
            "BC": 255, "CA": 255, "CB": 255, "CC": 255}
# drop plan (errsim wino_final: rel 1.71e-2): h-side lo of CC dropped fully,
# w-side lo of CC taps 0-3 dropped
H_KEEP = {s: list(range(PT)) for s in STREAMS}
H_KEEP["CC"] = []
W_KEEP = {s: list(range(PT)) for s in STREAMS}
W_KEEP["CC"] = [4, 5, 6, 7]
W_KEEP["CB"] = [2, 3, 4, 5, 6, 7]

F32 = mybir.dt.float32
BF16 = mybir.dt.bfloat16
E4 = mybir.dt.float8e4
E5 = mybir.dt.float8e5
E4np = ml_dtypes.float8_e4m3
E5np = ml_dtypes.float8_e5m2
BFnp = ml_dtypes.bfloat16

TRACE = False              # test.py flips this to get a profile
_cache = {}

DR = mybir.MatmulPerfMode.DoubleRow
LRELU = mybir.ActivationFunctionType.Prelu


def _build_wino():
    nc = bacc.Bacc("TRN2", target_bir_lowering=False, debug=False,
                   num_devices=NCORES)
    # x layout: col = (n*BPC + b)*FKT + kt
    xp_d = nc.dram_tensor("xp", [128, N * BPC * FKT], BF16,
                          kind="ExternalInput").ap()
    w_d = nc.dram_tensor("w", [4, 128, 4 * FKT * D], BF16,
                         kind="ExternalInput").ap()
    cwh_d = nc.dram_tensor("cwh", [4, 128, NS * PT * 2 * 128], E4,
                           kind="ExternalInput").ap()
    cwl_d = nc.dram_tensor("cwl", [4, 128, NS * PT * 2 * 128], E5,
                           kind="ExternalInput").ap()
    y_d = nc.dram_tensor("y", [BPC, O, T], BF16, kind="ExternalOutput").ap()

    with tile.TileContext(nc) as tc:
        with tc.tile_pool(name="x", bufs=4) as p_x, \
             tc.tile_pool(name="wg", bufs=4) as p_w, \
             tc.tile_pool(name="hd", bufs=1) as p_hd, \
             tc.tile_pool(name="leaf", bufs=1) as p_leaf, \
             tc.tile_pool(name="tmp", bufs=2) as p_tmp, \
             tc.tile_pool(name="cw", bufs=2) as p_cw, \
             tc.tile_pool(name="ep", bufs=4) as p_ep, \
             tc.tile_pool(name="yout", bufs=4) as p_out, \
             tc.tile_pool(name="ps1", bufs=2, space="PSUM") as p_ps1, \
             tc.tile_pool(name="ps2", bufs=6, space="PSUM") as p_ps2:

            # ---- tiles ----
            # x streamed through quarter-sized buffers (4 groups each)
            XQ = FKT * N * BPC // 4
            xq_sb = [p_x.tile([128, XQ], BF16, tag="x", name=f"xq{q}",
                              bufs=2) for q in range(4)]
            x_v = [t[:].rearrange("p (c kt) -> p c kt", kt=FKT)
                   for t in xq_sb]
            w_sb = [p_w.tile([128, 4 * FKT * D], BF16, tag="w", name=f"w{wt}")
                    for wt in range(4)]
            # h, D: [p, dh, n, b]
            h_sb = p_hd.tile([128, 2 * NB], BF16, tag="h", name="h_sb")
            d_sb = p_hd.tile([128, 2 * NB], BF16, tag="d", name="d_sb")
            hh = h_sb[:].rearrange("p (dh n b) -> p dh n b", dh=2, b=BPC)
            dd = d_sb[:].rearrange("p (dh n b) -> p dh n b", dh=2, b=BPC)
            # stride-4 views: n = 4*m + r
            hh4 = h_sb[:].rearrange("p (dh m r b) -> p dh m r b", dh=2, r=4,
                                    b=BPC)
            dd4 = d_sb[:].rearrange("p (dh m r b) -> p dh m r b", dh=2, r=4,
                                    b=BPC)
            # leaf streams: [p, s, dh, m, b]
            lf_hi = p_leaf.tile([128, NS * 2 * NL * BPC], E4, tag="lh",
                                name="lf_hi")
            # lo leaf: 8 streams only (CC's h-correction is dropped)
            lf_lo = p_leaf.tile([128, (NS - 1) * 2 * NL * BPC], E5, tag="ll",
                                name="lf_lo")
            lh = lf_hi[:].rearrange("p (s dh m b) -> p s dh m b", s=NS, dh=2,
                                    b=BPC)
            ll = lf_lo[:].rearrange("p (s dh m b) -> p s dh m b", s=NS - 1,
                                    dh=2, b=BPC)
            # conv weights, double-buffered by mo
            cwh_sb = [p_cw.tile([128, NS * PT * 2 * 128], E4, tag="cwh",
                                name=f"cwh{m}") for m in range(4)]
            cwl_sb = [p_cw.tile([128, NS * PT * 2 * 128], E5, tag="cwl",
                                name=f"cwl{m}") for m in range(4)]
            cwh_v = [t[:].rearrange("p (s tap dh o) -> p s tap dh o", s=NS,
                                    tap=PT, dh=2) for t in cwh_sb]
            cwl_v = [t[:].rearrange("p (s tap dh o) -> p s tap dh o", s=NS,
                                    tap=PT, dh=2) for t in cwl_sb]

            # ---- DMA emission (order == priority) ----
            def xchunk(q):
                nc.sync.dma_start(xq_sb[q][:], xp_d[:, q * XQ:(q + 1) * XQ])

            WH = 2 * FKT * D

            def wchunk(g2):
                wt, half = g2 // 2, g2 % 2
                nc.sync.dma_start(w_sb[wt][:, half * WH:(half + 1) * WH],
                                  w_d[wt, :, half * WH:(half + 1) * WH])

            # cw tiles are DMAed in 3-stream chunks so the conv can chase
            # the transfers (the DMA engine path is serial in practice).
            CWC = 3 * PT * 2 * 128                     # 3-stream chunk cols

            def cwchunk(kind, mo, c):
                src = cwh_d if kind == "h" else cwl_d
                dst = (cwh_sb if kind == "h" else cwl_sb)[mo]
                nc.sync.dma_start(dst[:, c * CWC:(c + 1) * CWC],
                                  src[mo, :, c * CWC:(c + 1) * CWC])

            # need-ordered: stage-1 feed (x, w) interleaved with mo0 conv
            # weights; later mo weights are emitted inside the conv loop so
            # y-out DMAs aren't stuck behind them.
            # cwh/cwl chunks interleaved per 3-stream group so mo0's T1
            # banks (stream-aligned) can close as soon as their chunk lands
            # first x quarter split in halves: group 0 is fed sooner
            nc.sync.dma_start(xq_sb[0][:, :XQ // 2], xp_d[:, :XQ // 2])
            wchunk(0)
            nc.sync.dma_start(xq_sb[0][:, XQ // 2:], xp_d[:, XQ // 2:XQ])
            wchunk(1)
            xchunk(1); wchunk(2); wchunk(3)           # groups 4-7
            cwchunk("h", 0, 0); cwchunk("l", 0, 0)
            xchunk(2); wchunk(4); wchunk(5)           # groups 8-11
            cwchunk("h", 0, 1); cwchunk("l", 0, 1)
            cwchunk("h", 0, 2); cwchunk("l", 0, 2)
            xchunk(3); wchunk(6); wchunk(7)           # groups 12-15
            for c in range(3):
                cwchunk("h", 1, c)

            # ---- PE warm-up ----
            warm_sb = p_hd.tile([128, 64], BF16, tag="warm", name="warm_sb",
                                bufs=1)
            nc.vector.memset(warm_sb[:], 0)

            def pewarm(n, label):
                ps = p_ps1.tile([128, BPC * S], F32, tag="ps1",
                                name=f"warm_{label}")
                for i in range(n):
                    nc.tensor.matmul(ps[:64, :64], warm_sb[:, :64],
                                     warm_sb[:, :64], start=True, stop=True)

            # ---- stage 1 ----
            def stage1(g):
                for m in range(2):
                    ps = p_ps1.tile([128, BPC * S], F32, tag="ps1",
                                    name=f"ps1_{g}_{m}")
                    for kt in range(FKT):
                        base = (g % 4) * FKT * D + kt * D + m * 128
                        lhsT = w_sb[g // 4][:, base: base + 128]
                        gq = g % 4
                        rhs = x_v[g // 4][:, gq * S * BPC:(gq + 1) * S * BPC,
                                          kt]
                        nc.tensor.matmul(ps[:], lhsT, rhs,
                                         start=(kt == 0), stop=(kt == FKT - 1))
                    psv = ps[:].rearrange("p (n b) -> p n b", b=BPC)
                    nc.scalar.copy(hh[:, m, g * S:(g + 1) * S, :], psv)

            # ---- transforms: D + 9 leaf streams, chunked over m ----
            # chunk c: m in [m0, m1); D over t in [4*m0, min(4*m1+6, 1023))
            def transforms(m0, m1):
                t0, t1 = 4 * m0, min(4 * m1 + 6, N - 1)
                nc.vector.tensor_tensor(dd[:, :, t0:t1, :],
                                        hh[:, :, t0:t1, :],
                                        hh[:, :, t0 + 1:t1 + 1, :],
                                        AluOpType.subtract)
                mw = m1 - m0

                def mview(v4, r, off=0):
                    # v4[:, :, m0+off : m1+off, r, :] clipped to valid range
                    return v4[:, :, m0 + off:m1 + off, r, :]

                # leaf sub specs: stream -> (src4, r_a, off_a, r_b, off_b)
                subs = {
                    "AB": (hh4, 1, 0, 3, 0), "AC": (hh4, 3, 0, 1, 1),
                    "BA": (hh4, 2, 0, 3, 0), "CA": (hh4, 3, 0, 0, 1),
                    "BB": (dd4, 0, 0, 2, 0), "BC": (dd4, 2, 0, 0, 1),
                    "CB": (dd4, 1, 0, 3, 0), "CC": (dd4, 3, 0, 1, 1),
                }
                # walrus vector/stt ops allow at most 2 free dims; the
                # stride-4 h/D views (dh, m, b) don't collapse, so ops that
                # read them are emitted once per dh half.
                # Engine split: subs on DVE; lo-corrections on Pool (SBUF
                # only, Pool is otherwise idle); hi casts on Act except the
                # first chunk (keeps the Act queue clear for stage-1 psum
                # copies, whose pool has only 2 buffers).
                def hi_eng_copy(out, in_):
                    nc.scalar.copy(out, in_)

                for s in STREAMS:
                    si = STREAMS.index(s)
                    me = min(m1, LEAF_LEN[s])
                    if me <= m0:
                        continue
                    w = me - m0
                    hi = lh[:, si, :, m0:me, :]
                    lo = (ll[:, si, :, m0:me, :] if si < NS - 1 else None)
                    if s == "AA":
                        for dh in range(2):
                            src = hh4[:, dh, m0:me, 3, :]
                            hi_eng_copy(hi[:, dh], src)
                            # split the lo halves across DVE and Pool
                            if dh == 0:
                                nc.vector.scalar_tensor_tensor(
                                    lo[:, dh], src, 1.0, hi[:, dh],
                                    AluOpType.mult, AluOpType.subtract)
                            else:
                                nc.gpsimd.tensor_tensor(
                                    lo[:, dh], src, hi[:, dh],
                                    AluOpType.subtract)
                        continue
                    v4, ra, oa, rb, ob = subs[s]
                    if s == "CC":
                        # h-corr dropped: quantize straight to e4m3
                        for dh in range(2):
                            nc.vector.tensor_tensor(
                                hi[:, dh], v4[:, dh, m0 + oa:me + oa, ra, :],
                                v4[:, dh, m0 + ob:me + ob, rb, :],
                                AluOpType.subtract)
                        continue
                    tmp = p_tmp.tile([128, 2 * 120 * BPC], BF16, tag="tmp",
                                     name=f"tmp_{s}_{m0}")
                    tv = tmp[:].rearrange("p (dh m b) -> p dh m b", dh=2,
                                          b=BPC)[:, :, :w, :]
                    for dh in range(2):
                        nc.vector.tensor_tensor(
                            tv[:, dh], v4[:, dh, m0 + oa:me + oa, ra, :],
                            v4[:, dh, m0 + ob:me + ob, rb, :],
                            AluOpType.subtract)
                    hi_eng_copy(hi, tv)
                    nc.vector.scalar_tensor_tensor(
                        lo[:, 0], tv[:, 0], 1.0, hi[:, 0],
                        AluOpType.mult, AluOpType.subtract)
                    nc.gpsimd.tensor_tensor(lo[:, 1], tv[:, 1], hi[:, 1],
                                            AluOpType.subtract)

            # ---- conv tiles ----
            # tile = (q0, w): j2 range [q0, q0+w). Stream s covers width
            # w if q0+w <= 248 else (w if y0-only stream else w-1)
            WIDE = {"AA", "AB", "BA", "BB"}     # used at j2 = 248 (phase 0)

            def swidth(s, q0, w):
                if q0 + w >= J2 and s not in WIDE:
                    return w - 1
                return w

            def conv_tile(mo, bb, q0, w, banks):
                """banks: list of lists of stream indices sharing one psum
                bank; stream k at col offset slot*(idx in bank)."""
                slot = 512 // max(len(bk) for bk in banks)
                pss = []
                for bi, bk in enumerate(banks):
                    ps = p_ps2.tile([128, 512], F32, tag="ps2",
                                    name=f"ps2_{mo}_{bb}_{q0}_{bi}")
                    ops = []
                    for k, si in enumerate(bk):
                        s = STREAMS[si]
                        ws_ = swidth(s, q0, w)
                        off = k * slot
                        for tap in range(PT):
                            ops.append((si, tap, "m", off, ws_))
                        for tap in H_KEEP[s]:
                            ops.append((si, tap, "h", off, ws_))
                        for tap in W_KEEP[s]:
                            ops.append((si, tap, "w", off, ws_))
                    for i, (si, tap, kind, off, ws_) in enumerate(ops):
                        if kind == "m":
                            l = cwh_v[mo][:, si, tap]
                            r = lh[:, si, :, q0 + tap:q0 + tap + ws_, bb]
                        elif kind == "h":
                            l = cwh_v[mo][:, si, tap]
                            r = ll[:, si, :, q0 + tap:q0 + tap + ws_, bb]
                        else:
                            l = cwl_v[mo][:, si, tap]
                            r = lh[:, si, :, q0 + tap:q0 + tap + ws_, bb]
                        nc.tensor.matmul(ps[:, off:off + ws_], l, r,
                                         start=(i == 0),
                                         stop=(i == len(ops) - 1),
                                         perf_mode=DR)
                    pss.append(ps)
                return pss, slot

            def epilogue_A(mo, bb, q0, w, banks, pss, slot, fast_tail=False):
                """PSUM-reading phase: copies of shared streams + partial
                adds. Running this promptly releases the unit's psum banks;
                the finals/activation (phase B) can lag behind."""
                def pr(si):
                    for bi, bk in enumerate(banks):
                        if si in bk:
                            off = bk.index(si) * slot
                            return pss[bi][:, off:off + swidth(
                                STREAMS[si], q0, w)]
                    raise KeyError(si)

                wn = w - 1 if q0 + w >= J2 else w   # narrow width (phases 1-3)
                AA, AB, AC = pr(0), pr(1), pr(2)
                BA, BB, BC = pr(3), pr(4), pr(5)
                CA, CB, CC = pr(6), pr(7), pr(8)
                # vector ops may read only ONE psum operand: copy the
                # shared streams (AA, BA, CA) to SBUF first, then each
                # partial add reads one psum + one sbuf input. Slot reuse:
                # u->t1, v->t3, w2->t5, then y0->t3, y1->t5, y2->t4,
                # y3->t6 (6 slots total).
                # bf16 partials: half the SBUF -> twice the pipeline depth
                tp = p_ep.tile([128, 6 * 128], BF16, tag="tp",
                               name=f"tp_{mo}_{bb}_{q0}")
                sl = [tp[:, k * 128:k * 128 + w] for k in range(6)]
                u, v, w2 = sl[0], sl[2], sl[4][:, :wn]
                nc.scalar.copy(u, AA)
                nc.scalar.copy(v, BA)
                nc.scalar.copy(w2, CA)
                t2 = sl[1][:, :wn]; t4 = sl[3][:, :wn]; t6 = sl[5][:, :wn]
                nc.vector.tensor_tensor(t2, u[:, :wn], AC, AluOpType.subtract)
                t1 = u
                nc.vector.tensor_tensor(t1, u, AB, AluOpType.add)
                nc.vector.tensor_tensor(t4, v[:, :wn], BC, AluOpType.subtract)
                t3 = v
                nc.vector.tensor_tensor(t3, v, BB, AluOpType.add)
                nc.vector.tensor_tensor(t6, w2, CC, AluOpType.subtract)
                t5 = w2
                nc.vector.tensor_tensor(t5, w2, CB, AluOpType.add)
                return (q0, w, wn, t1, t2, t3, t4, t5, t6, fast_tail)

            def epilogue_B(ctx, y_sb):
                q0, w, wn, t1, t2, t3, t4, t5, t6, fast_tail = ctx
                y0, y1, y2, y3 = t3, t5, t4, t6
                feng = nc.vector if fast_tail else nc.gpsimd
                feng.tensor_tensor(y0, t1, t3, AluOpType.add)
                feng.tensor_tensor(y1, t1[:, :wn], t5, AluOpType.subtract)
                nc.vector.tensor_tensor(y2, t2, t4, AluOpType.add)
                nc.vector.tensor_tensor(y3, t2, t6, AluOpType.subtract)
                yv = y_sb[:].rearrange("p (j2 r) -> p j2 r", r=4)
                nc.scalar.activation(yv[:, q0:q0 + w, 0], y0, LRELU,
                                     alpha=NEG_SLOPE)
                nc.scalar.activation(yv[:, q0:q0 + wn, 1], y1, LRELU,
                                     alpha=NEG_SLOPE)
                nc.scalar.activation(yv[:, q0:q0 + wn, 2], y2, LRELU,
                                     alpha=NEG_SLOPE)
                nc.scalar.activation(yv[:, q0:q0 + wn, 3], y3, LRELU,
                                     alpha=NEG_SLOPE)

            BANKS2 = [[0, 1, 2, 3, 4, 5, 6, 7], [8]]          # w <= 64
            BANKS3 = [[0, 1, 2], [3, 4, 5], [6, 7, 8]]        # chunk-aligned
            BANKS4 = [[0, 1, 2, 3], [4, 5, 6, 7], [8]]        # w <= 128
            BANKS5 = [[0, 1, 2, 3, 4], [5, 6, 7, 8]]          # w <= 96

            # phase-B (finals + activation) lags one sub-unit behind so the
            # next unit's psum-releasing phase-A ops aren't queued behind
            # cross-engine finals on the in-order engines.
            pendB = []

            def flushB(keep):
                while len(pendB) > keep:
                    pendB.pop(0)()

            def conv_unit(mo, bb, q0, w, banks, y_sb, fast_tail=False,
                          post=None):
                pss, slot = conv_tile(mo, bb, q0, w, banks)
                ctx = epilogue_A(mo, bb, q0, w, banks, pss, slot, fast_tail)

                def B():
                    epilogue_B(ctx, y_sb)
                    if post is not None:
                        post()
                pendB.append(B)
                flushB(1)

            # ---- schedule ----
            # transforms(c) is emitted after the last stage-1 group whose h it
            # needs, so the Act queue isn't blocked ahead of stage-1 copies.
            # pewarm bursts pad the DMA-starved windows (keeps the PE p-state
            # ramped so post-stall matmuls run at full clock).
            pewarm(70, "boot")
            for g in range(3):
                stage1(g)
            transforms(0, 40)          # needs h < 166  (g0-2)
            for g in range(3, 5):
                stage1(g)
            transforms(40, 72)         # needs h < 294  (g0-4)
            pewarm(20, "w0")
            for g in range(5, 7):
                stage1(g)
            transforms(72, 104)        # needs h < 422  (g0-6)
            for g in range(7, 9):
                stage1(g)
            transforms(104, 136)       # needs h < 550  (g0-8)
            pewarm(25, "w1")
            for g in range(9, 13):
                stage1(g)
            pewarm(25, "w2")
            for g in range(13, G):
                stage1(g)

            pewarm(40, "bridge")

            def ydma(mo, bb, y_sb):
                nc.sync.dma_start(y_d[bb, mo * 128:(mo + 1) * 128, :],
                                  y_sb[:, :T])

            # Unit schedule: T1 covers j2 [0,128) (two 64-wide units for
            # mo0, gated on transform chunks 0/1), T2 covers [128,249).
            # T2 units lag two T1 units behind so each unit's epilogue
            # drain (and psum-bank release) overlaps the next units'
            # matmuls, including across mo boundaries. mo0's last
            # transform chunk is emitted in 3 pieces between units so the
            # in-order DVE/Act queues interleave transform and epilogue
            # work; later cw DMAs are fed after the first unit of the
            # preceding mo.
            y_tiles = {}

            def emit_T1(mo, bb):
                y_sb = p_out.tile([128, 996], BF16, tag="y",
                                  name=f"y_{mo}_{bb}")
                y_tiles[(mo, bb)] = y_sb
                if mo == 0:
                    conv_unit(mo, bb, 0, 64, BANKS3, y_sb)
                    conv_unit(mo, bb, 64, 64, BANKS3, y_sb)
                    if bb < 3:
                        transforms(136 + 40 * bb, 176 + 40 * bb)
                    if bb == 0:
                        for c in range(3):
                            cwchunk("l", 1, c)
                else:
                    conv_unit(mo, bb, 0, 128, BANKS4, y_sb)
                    if bb == 0 and mo < 3:
                        for c in range(3):
                            cwchunk("h", mo + 1, c)
                            cwchunk("l", mo + 1, c)

            def emit_T2(mo, bb):
                y_sb = y_tiles[(mo, bb)]
                if mo == 3 and bb == BPC - 1:
                    # short tail: a small fast final tile + split y DMA
                    conv_unit(mo, bb, 128, 96, BANKS5, y_sb,
                              post=lambda: nc.sync.dma_start(
                                  y_d[bb, mo * 128:(mo + 1) * 128, :896],
                                  y_sb[:, :896]))
                    conv_unit(mo, bb, 224, 25, BANKS2, y_sb, fast_tail=True,
                              post=lambda: nc.sync.dma_start(
                                  y_d[bb, mo * 128:(mo + 1) * 128, 896:],
                                  y_sb[:, 896:T]))
                else:
                    conv_unit(mo, bb, 128, 121, BANKS4, y_sb,
                              post=lambda m=mo, b=bb, t=y_sb: ydma(m, b, t))

            # mo0's T1 units all run first (T2 needs the late transform
            # chunk); from mo1 on, T2 units lag two units behind.
            t2pend = []
            for mo in range(4):
                for bb in range(BPC):
                    emit_T1(mo, bb)
                    t2pend.append((mo, bb))
                    if mo == 3 and bb > 0:
                        # drain faster in the last mo so the final
                        # epilogue chains don't bunch at the end
                        emit_T2(*t2pend.pop(0))
                        if len(t2pend) > 1:
                            emit_T2(*t2pend.pop(0))
                    elif mo > 0 and len(t2pend) > 2:
                        emit_T2(*t2pend.pop(0))
                if mo == 0:
                    emit_T2(*t2pend.pop(0))
                    emit_T2(*t2pend.pop(0))
            for p in t2pend:
                emit_T2(*p)
            flushB(0)

    nc.compile()
    return nc


def _host_stage_wino(x, idx, W, conv_w):
    idx_flat = idx.reshape(-1).astype(np.int64)
    xg = x[:, idx_flat, :].astype(BFnp)                       # [B, N, F]
    xp = np.ascontiguousarray(
        xg.transpose(2, 1, 0).reshape(FKT, 128, N, NCORES, BPC)
        .transpose(3, 1, 2, 4, 0)).reshape(NCORES, 128, N * BPC * FKT)

    wq = np.ascontiguousarray(
        W.astype(BFnp).reshape(4, 4, FKT, 128, D).transpose(0, 3, 1, 2, 4)
    ).reshape(4, 128, 4 * FKT * D)

    # wino leaf weights: streams AA AB AC BA BB BC CA CB CC
    ws = (conv_w * np.float32(WSCALE)).astype(np.float32)     # [32, D, O]
    lvl1 = {"A": ws[0::2] + ws[1::2], "B": ws[0::2], "C": ws[1::2]}
    streams = []
    for s1 in "ABC":
        u = lvl1[s1]
        streams += [u[0::2] + u[1::2], u[0::2], u[1::2]]
    cws = np.stack(streams)                                   # [9, 8, D, O]
    # [s, tap, dh, p, mo, o] -> [mo, p, s, tap, dh, o]
    cwt = cws.reshape(NS, PT, 2, 128, 4, 128).transpose(4, 3, 0, 1, 2, 5)
    cwh = cwt.astype(E4np)
    cwl = (cwt - cwh.astype(np.float32)).astype(E5np)
    cwh = np.ascontiguousarray(cwh).reshape(4, 128, NS * PT * 2 * 128)
    cwl = np.ascontiguousarray(cwl).reshape(4, 128, NS * PT * 2 * 128)
    return xp, wq, cwh, cwl


def kernel(x, idx, W, b, conv_w, conv_b):
    x = np.asarray(x); idx = np.asarray(idx); W = np.asarray(W)
    b = np.asarray(b); conv_w = np.asarray(conv_w); conv_b = np.asarray(conv_b)
    has_bias = bool(np.any(b) or np.any(conv_b))
    if has_bias:
        # the harness always passes zero biases (spec fill=zeros); keep a
        # self-contained exact fallback for the general case
        xg = x[:, idx.reshape(-1).astype(np.int64), :]
        h = np.einsum('bgsf,gfd->bgsd',
                      xg.reshape(B, G, S, F).astype(np.float64),
                      W.astype(np.float64), optimize=True) + b[None, :, None]
        h = h.reshape(B, N, D)
        y = np.zeros((B, T, O), np.float64)
        for k in range(KC):
            y += np.einsum('btd,do->bto', h[:, k:k + T],
                           conv_w[k].astype(np.float64), optimize=True)
        y += conv_b[None, None, :]
        return np.where(y >= 0, y, NEG_SLOPE * y).astype(np.float32)

    if "wino" not in _cache:
        _cache["wino"] = _build_wino()
        _cache["nc"] = _cache["wino"]   # for test.py's TimelineSim hook
    nc = _cache["wino"]

    xp, wq, cwh, cwl = _host_stage_wino(x, idx, W, conv_w)
    in_maps = [{"xp": xp[c], "w": wq, "cwh": cwh, "cwl": cwl}
               for c in range(NCORES)]

    res = run_bass_kernel_spmd(nc, in_maps, core_ids=list(range(NCORES)),
                               trace=TRACE)
    if TRACE and res.exec_time_ns is not None:
        print(f"HW exec time: {res.exec_time_ns} ns")
    y = np.stack([r["y"] for r in res.results])       # [NC, BPC, O, T] bf16
    y = y.reshape(B, O, T).transpose(0, 2, 1).astype(np.float32)
    return np.ascontiguousarray(y * np.float32(1.0 / WSCALE))
